# revision 77
# baseline (speedup 1.0000x reference)
"""CWT (GMW filterbank) Trainium2 kernel, v4.

Computes Wx = ifft(Psih * fft(reflect_pad(x)))[..., N1:N1+L] for
x (32, 2048) f32, Psih (256, 4096) f32 -> out (32, 256, 2048) complex64.

Strategy (8 NeuronCores, data-parallel over batch, 4 rows/core),
optimized for SINGLE-SHOT execution time (prologue included):
  - Forward FFT via two-stage Cooley-Tukey (4096 = 32 x 128): fp16
    stage-1 DFT-128 matmul, DVE twiddle, PE transposes, stage-2
    block-diagonal matmul (ONE 3x64-col matmul per att component via
    pre-ordered plane triplets) producing xh {re, im, -im}; the 1/4096
    ifft normalization is folded into the stage-2 weights.
  - Banded mirror inverse DFT: per (octave, k-tile) pair (29 pairs at
    the 1e-2 band threshold) four products U = Pre@Er, W = Pre@Ei,
    -V = (-Pim)@Ei, Z = Pim@Er over the LEFT half n in [1024, 2048)
    only, shipped RAW to the host as fp16 (U,W)/(-V,Z) quad planes.
    The host reconstructs left = (U-V) + i(W+Z) and the mirrored right
    half = (U+V) + i(Z-W) for free, and computes the n=2048 center
    column from the shipped 24 KB xh spectrum.  This halves the output
    DMA (8.4 MB/core) and deletes all mirror/interleave DVE work and
    the old per-octave ctr matmuls.
  - fp16 throughout (E scaled to +-1, banded Psih 237 KB, x, xh, P,
    quads): same PE/DMA cost as bf16, ~8x finer quantization
    (global rel err ~5.7e-4).
  - DMA schedule: inputs (const packs, xs, banded psih, 16 E k-tiles
    kt-ascending = first-use order) stream on the SP HWDGE queue; the
    first 6 quad outputs also ride the SP queue BEHIND the E tiles
    (FIFO = input priority, no fence needed), later quads go via the
    gpsimd SWDGE queue, and the final unit's halves take the idle
    SP/ACT HWDGE queues so no SWDGE descriptor-gen trails the end.
  - Octave order [5,4,6,3,7,2,1,0]: wide-ish octaves early (slow PE
    consumption while E streams in), narrow octaves interleaved so
    their PSUM drains hide under wide-octave matmul stretches, widest
    (o0) last so a single quad trails the final matmul; that last unit
    is split into column halves so its copy+DMA overlaps its matmuls.
  - P-gen (P = Psih (.) xh, 3 DVE ops/octave) runs ahead per a fixed
    lookahead schedule so the big o1/o0 P tiles land in DVE slack.
  - PE p-state warmup: memset-fed transposes at t=0 ramp the PE clock
    while the first DMAs are still in flight.

Build notes (hard-won):
  - bacc.Bacc() + nc.compile() required (multi-wait legalization).
  - DVE tensor_tensor reads at most ONE operand from PSUM; free-dim APs
    capped at 3D; fp16 packed all-SBUF ops run 2x (broadcast-innermost
    operands drop back to 1x).
  - Matmul moving APs are capped at 512 elements (no [2,512] fusing).
  - PSUM is 8 banks; uv/wz [128,2,512] f32 tiles are 2 banks each,
    pool bufs=2 fills all 8; forward tiles are carved from the same
    tags via rotation.
Measured: TimelineSim single-shot 64.0 us (baseline v3: 119.7); HW
steady-state (loop-amortized) 71.5-72 us/iter (v3: 102); global rel
err 5.7e-4 (v3: 2.9e-3).  P-gen runs at DVE 2x via the batch-repeated
banded Psih (per-octave tiles DMA'd just-in-time within the E stream);
non-final quads ship as ONE 512 KB DMA per unit (E-tile pairing was
tried and REGRESSES: coarser arrival granularity stalls the early
octaves; per-kt E loads are the right grain).
"""

import numpy as np
import ml_dtypes

import concourse.bass as bass
import concourse.bacc as bacc
import concourse.mybir as mybir
import concourse.tile as tile
from concourse.bass_utils import run_bass_kernel_spmd

F16 = np.float16

B = 32          # batch
L = 2048        # signal length
UP = 4096       # padded length
N1 = 1024       # left pad (slice offset)
NA = 256        # scales
NV = 32         # voices/octave
NO = 8          # octaves
KF = 2048       # used frequency bins (Psih==0 at k=0 and k>=2048)
NC = 8          # cores
BPC = B // NC   # batch rows per core (4)
KT = KF // 128  # k tiles (16)
NTILE = 512     # output columns per matmul (left half = 2 tiles)
N1CT = 32       # CT inner length  (n = n1 + 32*n2)
N2CT = 128      # CT outer length

_CACHE = {}


def _bands_from(Psih):
    bands = []
    for o in range(NO):
        sub = np.asarray(Psih)[NV * o:NV * (o + 1), :KF]
        ks = np.nonzero((sub > 1e-2 * 2.0).any(axis=0))[0]
        bands.append((int(ks.min()) // 128, int(ks.max()) // 128 + 1))
    return bands


def _host_constants(Psih):
    """CT-FFT / inverse-DFT constant tensors + per-octave bands."""
    bands = _bands_from(Psih)

    # inverse DFT left half, NO 1/UP scale (folded into w32):
    # E[k, n] = exp(2i pi k n / UP), n in [N1, N1+L/2)
    kk = np.arange(KF)[:, None]
    nn = np.arange(N1, N1 + L // 2)[None, :]
    ph = 2.0 * np.pi * ((kk * nn) % UP) / UP
    # device layout: (kt, k_in 128, lnt, ri, n 512) fp16
    e_dev = np.empty((KT, 128, 2, 2, NTILE), dtype=F16)
    e_dev[:, :, :, 0, :] = np.cos(ph).reshape(KT, 128, 2, NTILE).astype(F16)
    e_dev[:, :, :, 1, :] = np.sin(ph).reshape(KT, 128, 2, NTILE).astype(F16)

    # stage-1 DFT-128 weights: W[n2, p] = exp(-2i pi n2 p / 128), fp16
    # (stage 1 runs fully in fp16: 1 cycle/row instead of 4)
    n2 = np.arange(N2CT)[:, None]
    p = np.arange(128)[None, :]
    w128_dev = np.empty((N2CT, 2, 128), dtype=F16)
    w128_dev[:, 0, :] = np.cos(2 * np.pi * n2 * p / N2CT).astype(F16)
    w128_dev[:, 1, :] = -np.sin(2 * np.pi * n2 * p / N2CT).astype(F16)

    # twiddle exp(-2i pi p n1 / UP): planes (cos, sin, -sin), f32
    pp = np.arange(128)[:, None]
    n1 = np.arange(N1CT)[None, :]
    tw_dev = np.empty((128, 3, N1CT), dtype=np.float32)
    tw_dev[:, 0, :] = np.cos(2 * np.pi * pp * n1 / UP)
    tw_dev[:, 1, :] = np.sin(2 * np.pi * pp * n1 / UP)
    tw_dev[:, 2, :] = -tw_dev[:, 1, :]

    # stage-2 block-diagonal rhs, scaled by 1/UP (ifft normalization):
    # R[(b',n1), plane, (b,q)] = (b'==b) * f(n1, q) / UP
    # planes ordered so each att component does ONE 3x64-col matmul:
    #   re-planes  (c, -s,  s) -> out blocks (Xre, Xim, -Xim)
    #   im-planes  (s,  c, -c)
    n1c = np.arange(N1CT)[:, None]
    qq = np.arange(KT)[None, :]
    c32 = np.cos(2 * np.pi * n1c * qq / N1CT) / UP
    s32 = np.sin(2 * np.pi * n1c * qq / N1CT) / UP
    w32_dev = np.zeros((BPC * N1CT, 6, BPC * KT), dtype=F16)
    for b in range(BPC):
        sl_r = slice(b * N1CT, (b + 1) * N1CT)
        sl_c = slice(b * KT, (b + 1) * KT)
        for pl, m in enumerate((c32, -s32, s32, s32, c32, -c32)):
            w32_dev[sl_r, pl, sl_c] = m.astype(F16)

    id128_dev = np.eye(128, dtype=np.float32)

    return e_dev, w128_dev, tw_dev, w32_dev, id128_dev, bands


def _pack_psihb(Psih, bands):
    """Banded Psih, fp16, repeated over the batch dim: [128 (k_in),
    sum(nk)*NV*BPC] with per-octave slices laid out [nk, NV, BPC]
    (b innermost) so every P-gen operand is packed fp16 -> DVE 2x."""
    tot = sum(hi - lo for lo, hi in bands)
    psihb = np.empty((128, tot * NV * BPC), dtype=F16)
    off = 0
    for o, (lo, hi) in enumerate(bands):
        nk = hi - lo
        # [nk, 128, NV] <- Psih[a, k].T slices
        blk = np.asarray(Psih)[NV * o:NV * (o + 1),
                               lo * 128:hi * 128].T.reshape(nk, 128, NV)
        rep = np.repeat(
            blk.transpose(1, 0, 2).reshape(128, nk * NV), BPC, axis=1
        )
        psihb[:, off * NV * BPC:(off + nk) * NV * BPC] = rep.astype(F16)
        off += nk
    return psihb


def _build_program(e_dev, w128_dev, tw_dev, w32_dev, id128_dev,
                   bands, reps=1, variant="full"):
    f32 = mybir.dt.float32
    f16 = mybir.dt.float16

    tot = sum(hi - lo for lo, hi in bands)
    offs = {}
    off = 0
    for o, (lo, hi) in enumerate(bands):
        offs[o] = off
        off += hi - lo

    nc = bacc.Bacc()
    xp_in = nc.dram_tensor("xp", [BPC, UP], f16, kind="ExternalInput")
    psihb_in = nc.dram_tensor("psihb", [128, tot * NV * BPC], f16,
                              kind="ExternalInput")
    outq_t = nc.dram_tensor("out_q", [NO, 2, 128, 4, NTILE], f16,
                            kind="ExternalOutput")
    xh_t = nc.dram_tensor("out_xh", [128, 3, KT, BPC], f16,
                          kind="ExternalOutput")

    e_c = nc.inline_tensor(e_dev, name="econst")
    c32_dev = np.concatenate(
        [id128_dev, tw_dev.reshape(128, 96)], axis=1
    )
    c16_dev = np.concatenate(
        [w128_dev.reshape(128, 256), w32_dev.reshape(128, 384)], axis=1
    )
    c32_c = nc.inline_tensor(c32_dev, name="c32const")
    c16_c = nc.inline_tensor(c16_dev, name="c16const")

    with tile.TileContext(nc) as tc:
        with (
            tc.tile_pool(name="persist", bufs=1) as persist,
            tc.tile_pool(name="pfix", bufs=1) as pfix,
            tc.tile_pool(name="fwd", bufs=2) as fwdp,
            tc.tile_pool(name="stg", bufs=16) as stgp,
            tc.tile_pool(name="stgf", bufs=2) as stgf,
            tc.tile_pool(name="ps_m", bufs=2, space="PSUM") as ps_m,
        ):
            # ---- PE p-state warmup: zeros tile via DVE memset (no DMA
            # dependency), then transposes keep PE busy and ramping while
            # the prologue DMAs stream in ----
            z_sb = persist.tile([128, 128], f32, tag="zwarm")
            nc.vector.memset(z_sb[:], 0.0)
            dummy = ps_m.tile([128, 2, NTILE], f32, tag="wz", name="dmy")
            for _ in range(12):
                nc.tensor.transpose(dummy[:, 0, 0:128], z_sb, z_sb)

            # ---- prologue: all input DMAs on the SP HWDGE queue, in
            # first-use order; small consts packed per dtype into single
            # DMAs; E k-tiles ascending (= first-use order for the
            # wide-early octave schedule). ----
            c16_sb = persist.tile([128, 640], f16, tag="c16")
            nc.sync.dma_start(out=c16_sb, in_=c16_c[:])
            w128_sb = c16_sb[:, 0:256].rearrange("p (r q) -> p r q", r=2)
            w32_sb = c16_sb[:, 256:640].rearrange("p (r q) -> p r q", r=6)
            xs_sb = persist.tile([N2CT, BPC * N1CT], f16, tag="xs")
            nc.sync.dma_start(
                out=xs_sb.rearrange("p (b m) -> p b m", b=BPC),
                in_=xp_in[:].rearrange("b (n2 n1) -> n2 b n1", n1=N1CT),
            )
            c32_sb = persist.tile([128, 224], f32, tag="c32")
            nc.sync.dma_start(out=c32_sb, in_=c32_c[:])
            id_sb = c32_sb[:, 0:128]
            tw_sb = c32_sb[:, 128:224].rearrange("p (r m) -> p r m", r=3)
            # per-octave psihb tiles, DMAs interleaved into the E
            # stream so each arrives just before its P-gen needs it
            # (separate tiles so a P-gen waits only its own slice)
            psih_tiles = {}

            def _psihb_dma(o):
                lo, hi = bands[o]
                nk = hi - lo
                offc = offs[o] * NV * BPC
                ptile = persist.tile([128, nk * NV * BPC], f16,
                                     tag=f"psihb{o}")
                nc.sync.dma_start(
                    out=ptile, in_=psihb_in[:, offc:offc + nk * NV * BPC]
                )
                psih_tiles[o] = ptile

            psihb_at = {-1: [5, 4], 0: [6, 3], 1: [7, 2], 2: [1], 3: [0]}
            for o in psihb_at[-1]:
                _psihb_dma(o)
            etiles = {}
            for kt in range(KT):
                et = persist.tile([128, 2, 2, NTILE], f16, tag=f"e{kt}")
                nc.sync.dma_start(out=et, in_=e_c[kt])
                etiles[kt] = et
                for o in psihb_at.get(kt, ()):
                    _psihb_dma(o)

            ctx = dict(
                nc=nc, bands=bands, offs=offs, outq_t=outq_t, xh_t=xh_t,
                z_sb=z_sb, dummy=dummy,
                persist=persist, pfix=pfix, fwdp=fwdp, stgp=stgp,
                stgf=stgf, ps_m=ps_m,
                psih_tiles=psih_tiles, xs_sb=xs_sb, w128_sb=w128_sb, tw_sb=tw_sb,
                w32_sb=w32_sb, id_sb=id_sb, etiles=etiles,
                f32=f32, f16=f16, variant=variant,
            )

            if reps == 1:
                _emit_body(ctx)
            else:
                with tc.For_i(0, reps, 1):
                    _emit_body(ctx)
    nc.compile()
    return nc


def _neg_comp(apx, n):
    """Same AP with dim 1 read in reverse order (indices n-1 .. 0)."""
    return bass.AP(
        apx.tensor,
        apx.offset + (n - 1) * apx.ap[1][0],
        [list(apx.ap[0]), [-apx.ap[1][0], n]] + [list(d) for d in apx.ap[2:]],
    )


def _emit_fwd(ctx):
    """Forward CT-FFT (4096 = 32 x 128): xh_all[p, {re,im,-im}, q, b] fp16,
    scaled by 1/UP (via w32)."""
    nc = ctx["nc"]
    f32, f16 = ctx["f32"], ctx["f16"]
    ps_m, fwdp, persist = ctx["ps_m"], ctx["fwdp"], ctx["persist"]
    xs_sb, w128_sb, tw_sb = ctx["xs_sb"], ctx["w128_sb"], ctx["tw_sb"]
    w32_sb, id_sb = ctx["w32_sb"], ctx["id_sb"]
    mult = mybir.AluOpType.mult

    # stage 1: A[p, (b, n1)] = sum_n2 xs[n2, (b, n1)] W128[n2, p], f32
    a_ps = ps_m.tile([128, 2, NTILE], f32, tag="uv", name="aps")
    for ri in range(2):
        nc.tensor.matmul(
            a_ps[:, ri, 0:BPC * N1CT], w128_sb[:, ri, :], xs_sb,
            start=True, stop=True,
        )
    # clock-bridge fillers: keep PE busy through the DVE twiddle window
    # so the transposes/stage-2/first inverse matmuls hit a hot p-state.
    # (Safe: the prologue dummy PSUM buffer is not reallocated until the
    # first inverse unit's wz tile, well after these complete.)
    for _ in range(9):
        nc.tensor.transpose(ctx["dummy"][:, 0, 0:128], ctx["z_sb"],
                            ctx["z_sb"])

    # twiddle At = A * exp(-2i pi p n1/4096), 3 DVE ops via -sin plane
    tmp = fwdp.tile([128, 4, BPC, N1CT], f32, tag="twtmp")
    at = fwdp.tile([128, 2, BPC * N1CT], f32, tag="at")
    a2 = a_ps[:, :, 0:BPC * N1CT].rearrange("p r (b m) -> p r b m", b=BPC)
    twc = tw_sb[:, 0, :][:, None, None, :].to_broadcast((128, 2, BPC, N1CT))
    tws = tw_sb[:, 1:3, :][:, :, None, :].to_broadcast((128, 2, BPC, N1CT))
    nc.vector.tensor_tensor(tmp[:, 0:2], a2, twc, mult)
    nc.vector.tensor_tensor(tmp[:, 2:4], a2, tws, mult)
    nc.vector.tensor_sub(
        at.rearrange("p r (b m) -> p r b m", b=BPC),
        tmp[:, 0:2], _neg_comp(tmp[:, 2:4], 2),
    )

    # transpose to [(b, n1), p]; round to fp16 for stage 2
    ta_ps = ps_m.tile([128, 2, NTILE], f32, tag="wz", name="taps")
    nc.tensor.transpose(ta_ps[:, 0, 0:128], at[:, 0, :], id_sb)
    nc.tensor.transpose(ta_ps[:, 1, 0:128], at[:, 1, :], id_sb)
    att = fwdp.tile([128, 2, 128], f16, tag="att")
    nc.vector.tensor_copy(out=att, in_=ta_ps[:, :, 0:128])

    # stage 2: XH[p, {re,im,-im}, (b, q)] -- two 3x64-col matmuls, one
    # per att component, using the pre-ordered w32 plane triplets
    xh_ps = ps_m.tile([128, 2, NTILE], f32, tag="uv", name="xhps")
    nq = BPC * KT
    nc.tensor.matmul(xh_ps[:, 0, 0:3 * nq], att[:, 0, :],
                     w32_sb[:, 0:3, :], start=True, stop=False)
    nc.tensor.matmul(xh_ps[:, 0, 0:3 * nq], att[:, 1, :],
                     w32_sb[:, 3:6, :], start=False, stop=True)
    # xh_all[p, comp, q, b] fp16 in SBUF for the P-gen broadcasts
    xh_all = persist.tile([128, 3, KT, BPC], f16, tag="xh")
    nc.vector.tensor_copy(
        out=xh_all,
        in_=xh_ps[:, 0, 0:3 * nq].rearrange("p (r b q) -> p r q b",
                                            r=3, b=BPC),
    )
    ctx["xh_all"] = xh_all
    ctx["xh_ps"] = xh_ps
    # ship the (tiny) spectrum: host computes the n=2048 center column
    # directly from it (emitted here, but the SP queue FIFO parks it
    # behind the E-tile loads, where it belongs)
    nc.sync.dma_start(out=ctx["xh_t"][:], in_=xh_all)


def _emit_pgen(ctx, o, eng=None, from_psum=False):
    """P[(o, kt in band, {re, im, -im})] = Psih (.) xh, 3 ops/octave.

    eng: engine to run on (default DVE); gpsimd for octaves generated
    while the Pool engine is otherwise idle.  from_psum: read xh straight
    from the stage-2 PSUM tile (skips the xh_all copy latency; only legal
    for the FIRST octave, before the PSUM tile rotates away)."""
    nc, bands, offs = ctx["nc"], ctx["bands"], ctx["offs"]
    pfix = ctx["pfix"]
    f16 = ctx["f16"]
    if eng is None:
        eng = nc.vector
    klo, khi = bands[o]
    nk = khi - klo
    # one tile PER COMPONENT so a matmul waits only the component it
    # reads (comp 2 generated first: it feeds the first matmul emitted
    # for single-ktile octaves)
    psih_ap = (
        ctx["psih_tiles"][o][:]
        .rearrange("p (k a b) -> p k a b", a=NV, b=BPC)
    )
    ptc = {}
    for comp in (2, 1, 0):
        pt = pfix.tile([128, nk, NV * BPC], f16, tag=f"P{o}c{comp}")
        out_ap = pt.rearrange("p k (a b) -> p k a b", b=BPC)
        xh_ap = (
            ctx["xh_all"][:, comp, klo:khi, None, :]
            .to_broadcast((128, nk, NV, BPC))
        )
        eng.tensor_tensor(out_ap, psih_ap, xh_ap, mybir.AluOpType.mult)
        ptc[comp] = pt
    ctx.setdefault("P", {})[o] = ptc


def _emit_body(ctx):
    """Forward + P-gen + banded quad inverse + quad output DMAs."""
    nc, bands = ctx["nc"], ctx["bands"]
    outq_t = ctx["outq_t"]
    stgp, stgf, ps_m = ctx["stgp"], ctx["stgf"], ctx["ps_m"]
    etiles = ctx["etiles"]
    f32, f16 = ctx["f32"], ctx["f16"]

    _emit_fwd(ctx)

    # Narrow octaves interleaved between wide ones so their copy+DMA
    # drains hide under wide-octave matmul stretches; widest (o0) last
    # so only one quad trails the final matmul.
    order = [5, 4, 6, 3, 7, 2, 1, 0]
    # P-gen runs ahead of the matmul stream; the big o1/o0 P tiles are
    # generated during wide octaves where DVE has slack.
    _emit_pgen(ctx, order[0])
    _emit_pgen(ctx, order[1])
    pgen_after = {0: [6], 1: [3], 2: [7], 3: [2], 4: [1], 5: [0]}

    ucnt = 0
    for oi, o in enumerate(order):
        klo, khi = bands[o]
        kts = list(range(klo, khi))
        ptc = ctx["P"][o]

        def P(comp, kt):
            return ptc[comp][:, kt - klo, :]

        # The very last unit is split into column halves so its copy+DMA
        # tail overlaps its own matmuls; its out-DMAs go on the SP/ACT
        # HWDGE queues (no SWDGE descriptor-gen serialization at the end).
        final = (oi == NO - 1)
        halves = ((slice(0, 256), slice(256, 512)) if final
                  else (slice(0, NTILE),))

        for lnt in range(2):
            for hs in (halves if (final and lnt == 1) else (slice(0, NTILE),)):
                # PSUM tiles pair products sharing the stationary weight:
                # uw = (U, W) from P0 (er then ei, one weight load on hw);
                # vz = (-V, Z) from P2/P1.  (A single [2,512] matmul is
                # illegal: matmul moving APs cap at 512 elements.)
                uv = ps_m.tile([128, 2, NTILE], f32, tag="uv")
                wz = ps_m.tile([128, 2, NTILE], f32, tag="wz")
                for j, kt in enumerate(kts):
                    first, last = (j == 0), (j == len(kts) - 1)
                    er = etiles[kt][:, lnt, 0, hs]
                    ei = etiles[kt][:, lnt, 1, hs]
                    if not last:
                        nc.tensor.matmul(uv[:, 0, hs], P(0, kt), er,
                                         start=first, stop=False)
                        nc.tensor.matmul(uv[:, 1, hs], P(0, kt), ei,
                                         start=first, stop=False)
                        nc.tensor.matmul(wz[:, 0, hs], P(2, kt), ei,
                                         start=first, stop=False)
                        nc.tensor.matmul(wz[:, 1, hs], P(1, kt), er,
                                         start=first, stop=False)
                    else:
                        # vz groups stop first so the slower DVE copy
                        # starts before the ACT one
                        nc.tensor.matmul(wz[:, 0, hs], P(2, kt), ei,
                                         start=first, stop=True)
                        nc.tensor.matmul(wz[:, 1, hs], P(1, kt), er,
                                         start=first, stop=True)
                        nc.tensor.matmul(uv[:, 0, hs], P(0, kt), er,
                                         start=first, stop=True)
                        nc.tensor.matmul(uv[:, 1, hs], P(0, kt), ei,
                                         start=first, stop=True)
                # quad (U,W,-V,Z) to SBUF fp16.  Non-final units ship as
                # ONE DMA per unit (fewer instructions; the 16-buf pool
                # hides the wait-for-both-copies); the final unit keeps
                # split halves so each ships as soon as its copy lands.
                # The first 6 units ship on the SP queue BEHIND the E
                # tiles (FIFO = input priority); later units go via the
                # gpsimd SWDGE queue (input stream nearly done by then).
                ucnt += 1
                if final and lnt == 1:
                    quv = stgf.tile([128, 2, NTILE], f16, tag="quv")
                    qwz = stgf.tile([128, 2, NTILE], f16, tag="qwz")
                    nc.scalar.copy(out=quv[:, :, hs], in_=uv[:, :, hs])
                    nc.scalar.dma_start(out=outq_t[o, lnt, :, 0:2, hs],
                                        in_=quv[:, :, hs])
                    nc.vector.tensor_copy(out=qwz[:, :, hs], in_=wz[:, :, hs])
                    nc.sync.dma_start(out=outq_t[o, lnt, :, 2:4, hs],
                                      in_=qwz[:, :, hs])
                else:
                    q = stgp.tile([128, 4, NTILE], f16, tag="quad")
                    nc.scalar.copy(out=q[:, 0:2, hs], in_=uv[:, :, hs])
                    nc.vector.tensor_copy(out=q[:, 2:4, hs], in_=wz[:, :, hs])
                    if ucnt <= 6:
                        nc.sync.dma_start(out=outq_t[o, lnt, :, :, hs],
                                          in_=q[:, :, hs])
                    else:
                        nc.gpsimd.dma_start(out=outq_t[o, lnt, :, :, hs],
                                            in_=q[:, :, hs])

        # P-gen for upcoming octaves per the lookahead schedule
        for oo in pgen_after.get(oi, ()):
            _emit_pgen(ctx, oo)


def _get_program(Psih, reps=1, variant="full"):
    key = f"prog{reps}_{variant}"
    if key not in _CACHE:
        if "consts" not in _CACHE:
            _CACHE["consts"] = _host_constants(np.asarray(Psih))
        (e_dev, w128_dev, tw_dev, w32_dev, id128_dev,
         bands) = _CACHE["consts"]
        nc = _build_program(e_dev, w128_dev, tw_dev, w32_dev,
                            id128_dev, bands, reps=reps, variant=variant)
        _CACHE[key] = (nc, bands)
    return _CACHE[key]


def _reflect_pad(x):
    return np.pad(x, ((0, 0), (N1, UP - L - N1)), mode="reflect")


_CTRW = {}


def _ctr_weight(Psih):
    """A[a, k] = Psih[a, k] * (-1)^k for the host-side n=2048 column."""
    if "w" not in _CTRW:
        sign = ((-1.0) ** (np.arange(KF) % 2)).astype(np.float32)
        _CTRW["w"] = np.asarray(Psih)[:, :KF].astype(np.float32) * sign
    return _CTRW["w"]


def _reconstruct(outq, xh, Psih):
    """Host-side: quads [NO, 2, 128, 4, 512] fp16 + spectrum
    xh [128, 3, KT, BPC] fp16 -> (BPC, NA, L) complex64 for one core."""
    oq = np.asarray(outq).astype(np.float32)
    # rows p = b*NV + a (b-major)
    oq = oq.reshape(NO, 2, NV, BPC, 4, NTILE)
    U = oq[:, :, :, :, 0]
    W = oq[:, :, :, :, 1]
    nV = oq[:, :, :, :, 2]
    Z = oq[:, :, :, :, 3]
    left = (U + nV) + 1j * (W + Z)        # [o, lnt, a, b, n]
    right = (U - nV) + 1j * (Z - W)
    # -> [b, o, a, lnt*512+n]
    left = left.transpose(3, 0, 2, 1, 4).reshape(BPC, NO * NV, L // 2)
    right = right.transpose(3, 0, 2, 1, 4).reshape(BPC, NO * NV, L // 2)
    out = np.empty((BPC, NA, L), dtype=np.complex64)
    out[:, :, 0:L // 2] = left
    # mirror: col 2048 - n2 for n2 in [1, 1024)
    out[:, :, L // 2 + 1:] = right[:, :, 1:][:, :, ::-1]
    # n=2048 center column from the shipped spectrum:
    # ctr[b, a] = sum_k Psih[a,k] * xh[b,k] * (-1)^k   (xh includes 1/UP)
    xh = np.asarray(xh).astype(np.float32)       # [p, comp, q, b]
    xhc = (xh[:, 0] + 1j * xh[:, 1]).transpose(2, 1, 0).reshape(BPC, KF)
    out[:, :, L // 2] = xhc @ _ctr_weight(Psih).T.astype(np.complex64)
    return out


def kernel(x, Psih=None, **_unused):
    x = np.ascontiguousarray(np.asarray(x), dtype=np.float32)
    if Psih is None:
        raise ValueError("Psih input required")
    nc, bands = _get_program(Psih)
    psihb = _pack_psihb(Psih, bands)
    xp = np.ascontiguousarray(_reflect_pad(x).astype(F16))
    in_maps = [
        {"xp": np.ascontiguousarray(xp[BPC * c:BPC * (c + 1)]),
         "psihb": psihb}
        for c in range(NC)
    ]
    res = run_bass_kernel_spmd(nc, in_maps, core_ids=list(range(NC)))
    out = np.concatenate(
        [_reconstruct(r["out_q"], r["out_xh"], Psih) for r in res.results],
        axis=0,
    )
    return out


def bench(x, Psih, iters=20, reps=1, variant="full"):
    """Run the kernel repeatedly on-device; returns (out_complex, times_ns).

    Builds the same shard_map executable as bass2jax.run_bass_via_pjrt but
    without donation, so the warm executable can be re-invoked with
    device-resident inputs."""
    import time
    import jax
    from jax.sharding import Mesh, PartitionSpec
    from jax.experimental.shard_map import shard_map
    from concourse import bass2jax

    x = np.ascontiguousarray(np.asarray(x), dtype=np.float32)
    nc, bands = _get_program(Psih, reps=reps, variant=variant)
    psihb = _pack_psihb(Psih, bands)
    bass2jax.install_neuronx_cc_hook()

    part_name = nc.partition_id_tensor.name if nc.partition_id_tensor else None
    in_names, out_names, out_avals = [], [], []
    for alloc in nc.m.functions[0].allocations:
        if not isinstance(alloc, mybir.MemoryLocationSet):
            continue
        name = alloc.memorylocations[0].name
        if alloc.kind == "ExternalInput":
            if name != part_name:
                in_names.append(name)
        elif alloc.kind == "ExternalOutput":
            out_names.append(name)
            out_avals.append(
                jax.core.ShapedArray(
                    tuple(alloc.tensor_shape), mybir.dt.np(alloc.dtype)
                )
            )
    n_params = len(in_names)
    all_names = in_names + out_names
    if part_name is not None:
        all_names = all_names + [part_name]

    def _body(*args):
        operands = list(args)
        if part_name is not None:
            operands.append(bass2jax.partition_id_tensor())
        outs = bass2jax._bass_exec_p.bind(
            *operands,
            out_avals=tuple(out_avals),
            in_names=tuple(all_names),
            out_names=tuple(out_names),
            lowering_input_output_aliases=(),
            sim_require_finite=True,
            sim_require_nnan=True,
            nc=nc,
        )
        return tuple(outs)

    devices = jax.devices()[:NC]
    mesh = Mesh(np.asarray(devices), ("core",))
    nin = n_params + len(out_names)
    fn = jax.jit(
        shard_map(
            _body,
            mesh=mesh,
            in_specs=(PartitionSpec("core"),) * nin,
            out_specs=(PartitionSpec("core"),) * len(out_names),
            check_rep=False,
        ),
        keep_unused=True,
    )
    xp = np.ascontiguousarray(_reflect_pad(x).astype(F16))
    in_map = {"xp": xp, "psihb": np.concatenate([psihb] * NC, axis=0)}
    concat_in = [in_map[n] for n in in_names]
    concat_zeros = [
        np.zeros((NC * a.shape[0], *a.shape[1:]), a.dtype) for a in out_avals
    ]
    sharding = jax.sharding.NamedSharding(mesh, PartitionSpec("core"))
    args = [jax.device_put(a, sharding) for a in concat_in + concat_zeros]
    out_arrs = jax.block_until_ready(fn(*args))  # compile + first run
    times = []
    for _ in range(iters):
        t0 = time.perf_counter()
        out_arrs = jax.block_until_ready(fn(*args))
        times.append((time.perf_counter() - t0) * 1e9)
    qname_i = out_names.index("out_q")
    xname_i = out_names.index("out_xh")
    oq = np.asarray(out_arrs[qname_i]).reshape(NC, NO, 2, 128, 4, NTILE)
    ox = np.asarray(out_arrs[xname_i]).reshape(NC, 128, 3, KT, BPC)
    out = np.concatenate(
        [_reconstruct(oq[c], ox[c], Psih) for c in range(NC)], axis=0
    )
    return out, times


# revision 79
# speedup vs baseline: 1.0284x; 1.0284x over previous
"""CWT (GMW filterbank) Trainium2 kernel, v4.

Computes Wx = ifft(Psih * fft(reflect_pad(x)))[..., N1:N1+L] for
x (32, 2048) f32, Psih (256, 4096) f32 -> out (32, 256, 2048) complex64.

Strategy (8 NeuronCores, data-parallel over batch, 4 rows/core),
optimized for SINGLE-SHOT execution time (prologue included):
  - Forward FFT via two-stage Cooley-Tukey (4096 = 32 x 128): fp16
    stage-1 DFT-128 matmul, DVE twiddle, PE transposes, stage-2
    block-diagonal matmul (ONE 3x64-col matmul per att component via
    pre-ordered plane triplets) producing xh {re, im, -im}; the 1/4096
    ifft normalization is folded into the stage-2 weights.
  - Banded mirror inverse DFT: per (octave, k-tile) pair (29 pairs at
    the 1e-2 band threshold) four products U = Pre@Er, W = Pre@Ei,
    -V = (-Pim)@Ei, Z = Pim@Er over the LEFT half n in [1024, 2048)
    only, shipped RAW to the host as fp16 (U,W)/(-V,Z) quad planes.
    The host reconstructs left = (U-V) + i(W+Z) and the mirrored right
    half = (U+V) + i(Z-W) for free, and computes the n=2048 center
    column from the shipped 24 KB xh spectrum.  This halves the output
    DMA (8.4 MB/core) and deletes all mirror/interleave DVE work and
    the old per-octave ctr matmuls.
  - fp16 throughout (E scaled to +-1, banded Psih 237 KB, x, xh, P,
    quads): same PE/DMA cost as bf16, ~8x finer quantization
    (global rel err ~5.7e-4).
  - DMA schedule: inputs (const packs, xs, banded psih, 16 E k-tiles
    kt-ascending = first-use order) stream on the SP HWDGE queue; the
    first 6 quad outputs also ride the SP queue BEHIND the E tiles
    (FIFO = input priority, no fence needed), later quads go via the
    gpsimd SWDGE queue, and the final unit's halves take the idle
    SP/ACT HWDGE queues so no SWDGE descriptor-gen trails the end.
  - Octave order [5,4,6,3,7,2,1,0]: wide-ish octaves early (slow PE
    consumption while E streams in), narrow octaves interleaved so
    their PSUM drains hide under wide-octave matmul stretches, widest
    (o0) last so a single quad trails the final matmul; that last unit
    is split into column halves so its copy+DMA overlaps its matmuls.
  - P-gen (P = Psih (.) xh, 3 DVE ops/octave) runs ahead per a fixed
    lookahead schedule so the big o1/o0 P tiles land in DVE slack.
  - PE p-state warmup: memset-fed transposes at t=0 ramp the PE clock
    while the first DMAs are still in flight.

Build notes (hard-won):
  - bacc.Bacc() + nc.compile() required (multi-wait legalization).
  - DVE tensor_tensor reads at most ONE operand from PSUM; free-dim APs
    capped at 3D; fp16 packed all-SBUF ops run 2x (broadcast-innermost
    operands drop back to 1x).
  - Matmul moving APs are capped at 512 elements (no [2,512] fusing).
  - PSUM is 8 banks; uv/wz [128,2,512] f32 tiles are 2 banks each,
    pool bufs=2 fills all 8; forward tiles are carved from the same
    tags via rotation.
Measured: TimelineSim single-shot 64.0 us (baseline v3: 119.7); HW
steady-state (loop-amortized) 71.5-72 us/iter (v3: 102); global rel
err 5.7e-4 (v3: 2.9e-3).  P-gen runs at DVE 2x via the batch-repeated
banded Psih (per-octave tiles DMA'd just-in-time within the E stream);
non-final quads ship as ONE 512 KB DMA per unit (E-tile pairing was
tried and REGRESSES: coarser arrival granularity stalls the early
octaves; per-kt E loads are the right grain).
"""

import numpy as np
import ml_dtypes

import concourse.bass as bass
import concourse.bacc as bacc
import concourse.mybir as mybir
import concourse.tile as tile
from concourse.bass_utils import run_bass_kernel_spmd

F16 = np.float16

B = 32          # batch
L = 2048        # signal length
UP = 4096       # padded length
N1 = 1024       # left pad (slice offset)
NA = 256        # scales
NV = 32         # voices/octave
NO = 8          # octaves
KF = 2048       # used frequency bins (Psih==0 at k=0 and k>=2048)
NC = 8          # cores
BPC = B // NC   # batch rows per core (4)
KT = KF // 128  # k tiles (16)
NTILE = 512     # output columns per matmul (left half = 2 tiles)
N1CT = 32       # CT inner length  (n = n1 + 32*n2)
N2CT = 128      # CT outer length

_CACHE = {}


def _bands_from(Psih):
    bands = []
    for o in range(NO):
        sub = np.asarray(Psih)[NV * o:NV * (o + 1), :KF]
        ks = np.nonzero((sub > 1e-2 * 2.0).any(axis=0))[0]
        bands.append((int(ks.min()) // 128, int(ks.max()) // 128 + 1))
    return bands


def _host_constants(Psih):
    """CT-FFT / inverse-DFT constant tensors + per-octave bands."""
    bands = _bands_from(Psih)

    # inverse DFT left half, NO 1/UP scale (folded into w32):
    # E[k, n] = exp(2i pi k n / UP), n in [N1, N1+L/2)
    kk = np.arange(KF)[:, None]
    nn = np.arange(N1, N1 + L // 2)[None, :]
    ph = 2.0 * np.pi * ((kk * nn) % UP) / UP
    # device layout: (kt, k_in 128, lnt, ri, n 512) fp16
    e_dev = np.empty((KT, 128, 2, 2, NTILE), dtype=F16)
    e_dev[:, :, :, 0, :] = np.cos(ph).reshape(KT, 128, 2, NTILE).astype(F16)
    e_dev[:, :, :, 1, :] = np.sin(ph).reshape(KT, 128, 2, NTILE).astype(F16)

    # stage-1 DFT-128 weights: W[n2, p] = exp(-2i pi n2 p / 128), fp16
    # (stage 1 runs fully in fp16: 1 cycle/row instead of 4)
    n2 = np.arange(N2CT)[:, None]
    p = np.arange(128)[None, :]
    w128_dev = np.empty((N2CT, 2, 128), dtype=F16)
    w128_dev[:, 0, :] = np.cos(2 * np.pi * n2 * p / N2CT).astype(F16)
    w128_dev[:, 1, :] = -np.sin(2 * np.pi * n2 * p / N2CT).astype(F16)

    # twiddle exp(-2i pi p n1 / UP): planes (cos, sin, -sin), f32
    pp = np.arange(128)[:, None]
    n1 = np.arange(N1CT)[None, :]
    tw_dev = np.empty((128, 3, N1CT), dtype=np.float32)
    tw_dev[:, 0, :] = np.cos(2 * np.pi * pp * n1 / UP)
    tw_dev[:, 1, :] = np.sin(2 * np.pi * pp * n1 / UP)
    tw_dev[:, 2, :] = -tw_dev[:, 1, :]

    # stage-2 block-diagonal rhs, scaled by 1/UP (ifft normalization):
    # R[(b',n1), plane, (b,q)] = (b'==b) * f(n1, q) / UP
    # planes ordered so each att component does ONE 3x64-col matmul:
    #   re-planes  (c, -s,  s) -> out blocks (Xre, Xim, -Xim)
    #   im-planes  (s,  c, -c)
    n1c = np.arange(N1CT)[:, None]
    qq = np.arange(KT)[None, :]
    c32 = np.cos(2 * np.pi * n1c * qq / N1CT) / UP
    s32 = np.sin(2 * np.pi * n1c * qq / N1CT) / UP
    w32_dev = np.zeros((BPC * N1CT, 6, BPC * KT), dtype=F16)
    for b in range(BPC):
        sl_r = slice(b * N1CT, (b + 1) * N1CT)
        sl_c = slice(b * KT, (b + 1) * KT)
        for pl, m in enumerate((c32, -s32, s32, s32, c32, -c32)):
            w32_dev[sl_r, pl, sl_c] = m.astype(F16)

    id128_dev = np.eye(128, dtype=np.float32)

    return e_dev, w128_dev, tw_dev, w32_dev, id128_dev, bands


def _pack_psihb(Psih, bands):
    """Banded Psih, fp16, repeated over the batch dim: [128 (k_in),
    sum(nk)*NV*BPC] with per-octave slices laid out [nk, NV, BPC]
    (b innermost) so every P-gen operand is packed fp16 -> DVE 2x."""
    tot = sum(hi - lo for lo, hi in bands)
    psihb = np.empty((128, tot * NV * BPC), dtype=F16)
    off = 0
    for o, (lo, hi) in enumerate(bands):
        nk = hi - lo
        # [nk, 128, NV] <- Psih[a, k].T slices
        blk = np.asarray(Psih)[NV * o:NV * (o + 1),
                               lo * 128:hi * 128].T.reshape(nk, 128, NV)
        rep = np.repeat(
            blk.transpose(1, 0, 2).reshape(128, nk * NV), BPC, axis=1
        )
        psihb[:, off * NV * BPC:(off + nk) * NV * BPC] = rep.astype(F16)
        off += nk
    return psihb


def _build_program(e_dev, w128_dev, tw_dev, w32_dev, id128_dev,
                   bands, reps=1, variant="full"):
    f32 = mybir.dt.float32
    f16 = mybir.dt.float16

    tot = sum(hi - lo for lo, hi in bands)
    offs = {}
    off = 0
    for o, (lo, hi) in enumerate(bands):
        offs[o] = off
        off += hi - lo

    nc = bacc.Bacc()
    xp_in = nc.dram_tensor("xp", [BPC, UP], f16, kind="ExternalInput")
    psihb_in = nc.dram_tensor("psihb", [128, tot * NV * BPC], f16,
                              kind="ExternalInput")
    outq_t = nc.dram_tensor("out_q", [NO, 2, 128, 4, NTILE], f16,
                            kind="ExternalOutput")
    xh_t = nc.dram_tensor("out_xh", [128, 3, KT, BPC], f16,
                          kind="ExternalOutput")

    e_c = nc.inline_tensor(e_dev, name="econst")
    c32_dev = np.concatenate(
        [id128_dev, tw_dev.reshape(128, 96)], axis=1
    )
    c16_dev = np.concatenate(
        [w128_dev.reshape(128, 256), w32_dev.reshape(128, 384)], axis=1
    )
    c32_c = nc.inline_tensor(c32_dev, name="c32const")
    c16_c = nc.inline_tensor(c16_dev, name="c16const")

    with tile.TileContext(nc) as tc:
        with (
            tc.tile_pool(name="persist", bufs=1) as persist,
            tc.tile_pool(name="pfix", bufs=1) as pfix,
            tc.tile_pool(name="fwd", bufs=2) as fwdp,
            tc.tile_pool(name="stg", bufs=16) as stgp,
            tc.tile_pool(name="stgf", bufs=2) as stgf,
            tc.tile_pool(name="ps_m", bufs=2, space="PSUM") as ps_m,
        ):
            # ---- PE p-state warmup: zeros tile via DVE memset (no DMA
            # dependency), then transposes keep PE busy and ramping while
            # the prologue DMAs stream in ----
            z_sb = persist.tile([128, 128], f32, tag="zwarm")
            nc.vector.memset(z_sb[:], 0.0)
            dummy = ps_m.tile([128, 2, NTILE], f32, tag="wz", name="dmy")
            # 6 only: on real HW these cost ~440 ns each with no
            # measurable DVFS benefit (measured via in-loop fillers), so
            # keep the warmup safely inside the first-DMA wait window
            for _ in range(6):
                nc.tensor.transpose(dummy[:, 0, 0:128], z_sb, z_sb)

            # ---- prologue: all input DMAs on the SP HWDGE queue, in
            # first-use order; small consts packed per dtype into single
            # DMAs; E k-tiles ascending (= first-use order for the
            # wide-early octave schedule). ----
            c16_sb = persist.tile([128, 640], f16, tag="c16")
            nc.sync.dma_start(out=c16_sb, in_=c16_c[:])
            w128_sb = c16_sb[:, 0:256].rearrange("p (r q) -> p r q", r=2)
            w32_sb = c16_sb[:, 256:640].rearrange("p (r q) -> p r q", r=6)
            xs_sb = persist.tile([N2CT, BPC * N1CT], f16, tag="xs")
            nc.sync.dma_start(
                out=xs_sb.rearrange("p (b m) -> p b m", b=BPC),
                in_=xp_in[:].rearrange("b (n2 n1) -> n2 b n1", n1=N1CT),
            )
            c32_sb = persist.tile([128, 224], f32, tag="c32")
            nc.sync.dma_start(out=c32_sb, in_=c32_c[:])
            id_sb = c32_sb[:, 0:128]
            tw_sb = c32_sb[:, 128:224].rearrange("p (r m) -> p r m", r=3)
            # per-octave psihb tiles, DMAs interleaved into the E
            # stream so each arrives just before its P-gen needs it
            # (separate tiles so a P-gen waits only its own slice)
            psih_tiles = {}

            def _psihb_dma(o):
                lo, hi = bands[o]
                nk = hi - lo
                offc = offs[o] * NV * BPC
                ptile = persist.tile([128, nk * NV * BPC], f16,
                                     tag=f"psihb{o}")
                nc.sync.dma_start(
                    out=ptile, in_=psihb_in[:, offc:offc + nk * NV * BPC]
                )
                psih_tiles[o] = ptile

            psihb_at = {-1: [5, 4], 0: [6, 3], 1: [7, 2], 2: [1], 3: [0]}
            for o in psihb_at[-1]:
                _psihb_dma(o)
            etiles = {}
            for kt in range(KT):
                et = persist.tile([128, 2, 2, NTILE], f16, tag=f"e{kt}")
                nc.sync.dma_start(out=et, in_=e_c[kt])
                etiles[kt] = et
                for o in psihb_at.get(kt, ()):
                    _psihb_dma(o)

            ctx = dict(
                nc=nc, bands=bands, offs=offs, outq_t=outq_t, xh_t=xh_t,
                z_sb=z_sb, dummy=dummy,
                persist=persist, pfix=pfix, fwdp=fwdp, stgp=stgp,
                stgf=stgf, ps_m=ps_m,
                psih_tiles=psih_tiles, xs_sb=xs_sb, w128_sb=w128_sb, tw_sb=tw_sb,
                w32_sb=w32_sb, id_sb=id_sb, etiles=etiles,
                f32=f32, f16=f16, variant=variant,
            )

            if reps == 1:
                _emit_body(ctx)
            else:
                with tc.For_i(0, reps, 1):
                    _emit_body(ctx)
    nc.compile()
    return nc


def _neg_comp(apx, n):
    """Same AP with dim 1 read in reverse order (indices n-1 .. 0)."""
    return bass.AP(
        apx.tensor,
        apx.offset + (n - 1) * apx.ap[1][0],
        [list(apx.ap[0]), [-apx.ap[1][0], n]] + [list(d) for d in apx.ap[2:]],
    )


def _emit_fwd(ctx):
    """Forward CT-FFT (4096 = 32 x 128): xh_all[p, {re,im,-im}, q, b] fp16,
    scaled by 1/UP (via w32)."""
    nc = ctx["nc"]
    f32, f16 = ctx["f32"], ctx["f16"]
    ps_m, fwdp, persist = ctx["ps_m"], ctx["fwdp"], ctx["persist"]
    xs_sb, w128_sb, tw_sb = ctx["xs_sb"], ctx["w128_sb"], ctx["tw_sb"]
    w32_sb, id_sb = ctx["w32_sb"], ctx["id_sb"]
    mult = mybir.AluOpType.mult

    # stage 1: A[p, (b, n1)] = sum_n2 xs[n2, (b, n1)] W128[n2, p], f32
    a_ps = ps_m.tile([128, 2, NTILE], f32, tag="uv", name="aps")
    for ri in range(2):
        nc.tensor.matmul(
            a_ps[:, ri, 0:BPC * N1CT], w128_sb[:, ri, :], xs_sb,
            start=True, stop=True,
        )

    # twiddle At = A * exp(-2i pi p n1/4096), 3 DVE ops via -sin plane
    tmp = fwdp.tile([128, 4, BPC, N1CT], f32, tag="twtmp")
    at = fwdp.tile([128, 2, BPC * N1CT], f32, tag="at")
    a2 = a_ps[:, :, 0:BPC * N1CT].rearrange("p r (b m) -> p r b m", b=BPC)
    twc = tw_sb[:, 0, :][:, None, None, :].to_broadcast((128, 2, BPC, N1CT))
    tws = tw_sb[:, 1:3, :][:, :, None, :].to_broadcast((128, 2, BPC, N1CT))
    nc.vector.tensor_tensor(tmp[:, 0:2], a2, twc, mult)
    nc.vector.tensor_tensor(tmp[:, 2:4], a2, tws, mult)
    nc.vector.tensor_sub(
        at.rearrange("p r (b m) -> p r b m", b=BPC),
        tmp[:, 0:2], _neg_comp(tmp[:, 2:4], 2),
    )

    # transpose to [(b, n1), p]; round to fp16 for stage 2
    ta_ps = ps_m.tile([128, 2, NTILE], f32, tag="wz", name="taps")
    nc.tensor.transpose(ta_ps[:, 0, 0:128], at[:, 0, :], id_sb)
    nc.tensor.transpose(ta_ps[:, 1, 0:128], at[:, 1, :], id_sb)
    att = fwdp.tile([128, 2, 128], f16, tag="att")
    nc.vector.tensor_copy(out=att, in_=ta_ps[:, :, 0:128])

    # stage 2: XH[p, {re,im,-im}, (b, q)] -- two 3x64-col matmuls, one
    # per att component, using the pre-ordered w32 plane triplets
    xh_ps = ps_m.tile([128, 2, NTILE], f32, tag="uv", name="xhps")
    nq = BPC * KT
    nc.tensor.matmul(xh_ps[:, 0, 0:3 * nq], att[:, 0, :],
                     w32_sb[:, 0:3, :], start=True, stop=False)
    nc.tensor.matmul(xh_ps[:, 0, 0:3 * nq], att[:, 1, :],
                     w32_sb[:, 3:6, :], start=False, stop=True)
    # xh_all[p, comp, q, b] fp16 in SBUF for the P-gen broadcasts
    xh_all = persist.tile([128, 3, KT, BPC], f16, tag="xh")
    nc.vector.tensor_copy(
        out=xh_all,
        in_=xh_ps[:, 0, 0:3 * nq].rearrange("p (r b q) -> p r q b",
                                            r=3, b=BPC),
    )
    ctx["xh_all"] = xh_all
    ctx["xh_ps"] = xh_ps
    # ship the (tiny) spectrum: host computes the n=2048 center column
    # directly from it (emitted here, but the SP queue FIFO parks it
    # behind the E-tile loads, where it belongs)
    nc.sync.dma_start(out=ctx["xh_t"][:], in_=xh_all)


def _emit_pgen(ctx, o, eng=None, from_psum=False):
    """P[(o, kt in band, {re, im, -im})] = Psih (.) xh, 3 ops/octave.

    eng: engine to run on (default DVE); gpsimd for octaves generated
    while the Pool engine is otherwise idle.  from_psum: read xh straight
    from the stage-2 PSUM tile (skips the xh_all copy latency; only legal
    for the FIRST octave, before the PSUM tile rotates away)."""
    nc, bands, offs = ctx["nc"], ctx["bands"], ctx["offs"]
    pfix = ctx["pfix"]
    f16 = ctx["f16"]
    if eng is None:
        eng = nc.vector
    klo, khi = bands[o]
    nk = khi - klo
    # one tile PER COMPONENT so a matmul waits only the component it
    # reads (comp 2 generated first: it feeds the first matmul emitted
    # for single-ktile octaves)
    psih_ap = (
        ctx["psih_tiles"][o][:]
        .rearrange("p (k a b) -> p k a b", a=NV, b=BPC)
    )
    ptc = {}
    for comp in (2, 1, 0):
        pt = pfix.tile([128, nk, NV * BPC], f16, tag=f"P{o}c{comp}")
        out_ap = pt.rearrange("p k (a b) -> p k a b", b=BPC)
        xh_ap = (
            ctx["xh_all"][:, comp, klo:khi, None, :]
            .to_broadcast((128, nk, NV, BPC))
        )
        eng.tensor_tensor(out_ap, psih_ap, xh_ap, mybir.AluOpType.mult)
        ptc[comp] = pt
    ctx.setdefault("P", {})[o] = ptc


def _emit_body(ctx):
    """Forward + P-gen + banded quad inverse + quad output DMAs."""
    nc, bands = ctx["nc"], ctx["bands"]
    outq_t = ctx["outq_t"]
    stgp, stgf, ps_m = ctx["stgp"], ctx["stgf"], ctx["ps_m"]
    etiles = ctx["etiles"]
    f32, f16 = ctx["f32"], ctx["f16"]

    _emit_fwd(ctx)

    # Narrow octaves interleaved between wide ones so their copy+DMA
    # drains hide under wide-octave matmul stretches; widest (o0) last
    # so only one quad trails the final matmul.
    order = [5, 4, 6, 3, 7, 2, 1, 0]
    # P-gen runs ahead of the matmul stream; the big o1/o0 P tiles are
    # generated during wide octaves where DVE has slack.
    _emit_pgen(ctx, order[0])
    _emit_pgen(ctx, order[1])
    pgen_after = {0: [6], 1: [3], 2: [7], 3: [2], 4: [1], 5: [0]}

    ucnt = 0
    for oi, o in enumerate(order):
        klo, khi = bands[o]
        kts = list(range(klo, khi))
        ptc = ctx["P"][o]

        def P(comp, kt):
            return ptc[comp][:, kt - klo, :]

        # The very last unit is split into column halves so its copy+DMA
        # tail overlaps its own matmuls; its out-DMAs go on the SP/ACT
        # HWDGE queues (no SWDGE descriptor-gen serialization at the end).
        final = (oi == NO - 1)
        halves = ((slice(0, 256), slice(256, 512)) if final
                  else (slice(0, NTILE),))

        for lnt in range(2):
            for hs in (halves if (final and lnt == 1) else (slice(0, NTILE),)):
                # PSUM tiles pair products sharing the stationary weight:
                # uw = (U, W) from P0 (er then ei, one weight load on hw);
                # vz = (-V, Z) from P2/P1.  (A single [2,512] matmul is
                # illegal: matmul moving APs cap at 512 elements.)
                uv = ps_m.tile([128, 2, NTILE], f32, tag="uv")
                wz = ps_m.tile([128, 2, NTILE], f32, tag="wz")
                for j, kt in enumerate(kts):
                    first, last = (j == 0), (j == len(kts) - 1)
                    er = etiles[kt][:, lnt, 0, hs]
                    ei = etiles[kt][:, lnt, 1, hs]
                    if not last:
                        nc.tensor.matmul(uv[:, 0, hs], P(0, kt), er,
                                         start=first, stop=False)
                        nc.tensor.matmul(uv[:, 1, hs], P(0, kt), ei,
                                         start=first, stop=False)
                        nc.tensor.matmul(wz[:, 0, hs], P(2, kt), ei,
                                         start=first, stop=False)
                        nc.tensor.matmul(wz[:, 1, hs], P(1, kt), er,
                                         start=first, stop=False)
                    else:
                        # vz groups stop first so the slower DVE copy
                        # starts before the ACT one
                        nc.tensor.matmul(wz[:, 0, hs], P(2, kt), ei,
                                         start=first, stop=True)
                        nc.tensor.matmul(wz[:, 1, hs], P(1, kt), er,
                                         start=first, stop=True)
                        nc.tensor.matmul(uv[:, 0, hs], P(0, kt), er,
                                         start=first, stop=True)
                        nc.tensor.matmul(uv[:, 1, hs], P(0, kt), ei,
                                         start=first, stop=True)
                # quad (U,W,-V,Z) to SBUF fp16.  Non-final units ship as
                # ONE DMA per unit (fewer instructions; the 16-buf pool
                # hides the wait-for-both-copies); the final unit keeps
                # split halves so each ships as soon as its copy lands.
                # The first 6 units ship on the SP queue BEHIND the E
                # tiles (FIFO = input priority); later units go via the
                # gpsimd SWDGE queue (input stream nearly done by then).
                ucnt += 1
                if final and lnt == 1:
                    quv = stgf.tile([128, 2, NTILE], f16, tag="quv")
                    qwz = stgf.tile([128, 2, NTILE], f16, tag="qwz")
                    nc.scalar.copy(out=quv[:, :, hs], in_=uv[:, :, hs])
                    nc.scalar.dma_start(out=outq_t[o, lnt, :, 0:2, hs],
                                        in_=quv[:, :, hs])
                    nc.vector.tensor_copy(out=qwz[:, :, hs], in_=wz[:, :, hs])
                    nc.sync.dma_start(out=outq_t[o, lnt, :, 2:4, hs],
                                      in_=qwz[:, :, hs])
                else:
                    q = stgp.tile([128, 4, NTILE], f16, tag="quad")
                    nc.scalar.copy(out=q[:, 0:2, hs], in_=uv[:, :, hs])
                    nc.vector.tensor_copy(out=q[:, 2:4, hs], in_=wz[:, :, hs])
                    if ucnt <= 6:
                        nc.sync.dma_start(out=outq_t[o, lnt, :, :, hs],
                                          in_=q[:, :, hs])
                    else:
                        nc.gpsimd.dma_start(out=outq_t[o, lnt, :, :, hs],
                                            in_=q[:, :, hs])

        # P-gen for upcoming octaves per the lookahead schedule
        for oo in pgen_after.get(oi, ()):
            _emit_pgen(ctx, oo)


def _get_program(Psih, reps=1, variant="full"):
    key = f"prog{reps}_{variant}"
    if key not in _CACHE:
        if "consts" not in _CACHE:
            _CACHE["consts"] = _host_constants(np.asarray(Psih))
        (e_dev, w128_dev, tw_dev, w32_dev, id128_dev,
         bands) = _CACHE["consts"]
        nc = _build_program(e_dev, w128_dev, tw_dev, w32_dev,
                            id128_dev, bands, reps=reps, variant=variant)
        _CACHE[key] = (nc, bands)
    return _CACHE[key]


def _reflect_pad(x):
    return np.pad(x, ((0, 0), (N1, UP - L - N1)), mode="reflect")


_CTRW = {}


def _ctr_weight(Psih):
    """A[a, k] = Psih[a, k] * (-1)^k for the host-side n=2048 column."""
    if "w" not in _CTRW:
        sign = ((-1.0) ** (np.arange(KF) % 2)).astype(np.float32)
        _CTRW["w"] = np.asarray(Psih)[:, :KF].astype(np.float32) * sign
    return _CTRW["w"]


def _reconstruct(outq, xh, Psih):
    """Host-side: quads [NO, 2, 128, 4, 512] fp16 + spectrum
    xh [128, 3, KT, BPC] fp16 -> (BPC, NA, L) complex64 for one core."""
    oq = np.asarray(outq).astype(np.float32)
    # rows p = b*NV + a (b-major)
    oq = oq.reshape(NO, 2, NV, BPC, 4, NTILE)
    U = oq[:, :, :, :, 0]
    W = oq[:, :, :, :, 1]
    nV = oq[:, :, :, :, 2]
    Z = oq[:, :, :, :, 3]
    left = (U + nV) + 1j * (W + Z)        # [o, lnt, a, b, n]
    right = (U - nV) + 1j * (Z - W)
    # -> [b, o, a, lnt*512+n]
    left = left.transpose(3, 0, 2, 1, 4).reshape(BPC, NO * NV, L // 2)
    right = right.transpose(3, 0, 2, 1, 4).reshape(BPC, NO * NV, L // 2)
    out = np.empty((BPC, NA, L), dtype=np.complex64)
    out[:, :, 0:L // 2] = left
    # mirror: col 2048 - n2 for n2 in [1, 1024)
    out[:, :, L // 2 + 1:] = right[:, :, 1:][:, :, ::-1]
    # n=2048 center column from the shipped spectrum:
    # ctr[b, a] = sum_k Psih[a,k] * xh[b,k] * (-1)^k   (xh includes 1/UP)
    xh = np.asarray(xh).astype(np.float32)       # [p, comp, q, b]
    xhc = (xh[:, 0] + 1j * xh[:, 1]).transpose(2, 1, 0).reshape(BPC, KF)
    out[:, :, L // 2] = xhc @ _ctr_weight(Psih).T.astype(np.complex64)
    return out


def kernel(x, Psih=None, **_unused):
    x = np.ascontiguousarray(np.asarray(x), dtype=np.float32)
    if Psih is None:
        raise ValueError("Psih input required")
    nc, bands = _get_program(Psih)
    psihb = _pack_psihb(Psih, bands)
    xp = np.ascontiguousarray(_reflect_pad(x).astype(F16))
    in_maps = [
        {"xp": np.ascontiguousarray(xp[BPC * c:BPC * (c + 1)]),
         "psihb": psihb}
        for c in range(NC)
    ]
    res = run_bass_kernel_spmd(nc, in_maps, core_ids=list(range(NC)))
    out = np.concatenate(
        [_reconstruct(r["out_q"], r["out_xh"], Psih) for r in res.results],
        axis=0,
    )
    return out


def bench(x, Psih, iters=20, reps=1, variant="full"):
    """Run the kernel repeatedly on-device; returns (out_complex, times_ns).

    Builds the same shard_map executable as bass2jax.run_bass_via_pjrt but
    without donation, so the warm executable can be re-invoked with
    device-resident inputs."""
    import time
    import jax
    from jax.sharding import Mesh, PartitionSpec
    from jax.experimental.shard_map import shard_map
    from concourse import bass2jax

    x = np.ascontiguousarray(np.asarray(x), dtype=np.float32)
    nc, bands = _get_program(Psih, reps=reps, variant=variant)
    psihb = _pack_psihb(Psih, bands)
    bass2jax.install_neuronx_cc_hook()

    part_name = nc.partition_id_tensor.name if nc.partition_id_tensor else None
    in_names, out_names, out_avals = [], [], []
    for alloc in nc.m.functions[0].allocations:
        if not isinstance(alloc, mybir.MemoryLocationSet):
            continue
        name = alloc.memorylocations[0].name
        if alloc.kind == "ExternalInput":
            if name != part_name:
                in_names.append(name)
        elif alloc.kind == "ExternalOutput":
            out_names.append(name)
            out_avals.append(
                jax.core.ShapedArray(
                    tuple(alloc.tensor_shape), mybir.dt.np(alloc.dtype)
                )
            )
    n_params = len(in_names)
    all_names = in_names + out_names
    if part_name is not None:
        all_names = all_names + [part_name]

    def _body(*args):
        operands = list(args)
        if part_name is not None:
            operands.append(bass2jax.partition_id_tensor())
        outs = bass2jax._bass_exec_p.bind(
            *operands,
            out_avals=tuple(out_avals),
            in_names=tuple(all_names),
            out_names=tuple(out_names),
            lowering_input_output_aliases=(),
            sim_require_finite=True,
            sim_require_nnan=True,
            nc=nc,
        )
        return tuple(outs)

    devices = jax.devices()[:NC]
    mesh = Mesh(np.asarray(devices), ("core",))
    nin = n_params + len(out_names)
    fn = jax.jit(
        shard_map(
            _body,
            mesh=mesh,
            in_specs=(PartitionSpec("core"),) * nin,
            out_specs=(PartitionSpec("core"),) * len(out_names),
            check_rep=False,
        ),
        keep_unused=True,
    )
    xp = np.ascontiguousarray(_reflect_pad(x).astype(F16))
    in_map = {"xp": xp, "psihb": np.concatenate([psihb] * NC, axis=0)}
    concat_in = [in_map[n] for n in in_names]
    concat_zeros = [
        np.zeros((NC * a.shape[0], *a.shape[1:]), a.dtype) for a in out_avals
    ]
    sharding = jax.sharding.NamedSharding(mesh, PartitionSpec("core"))
    args = [jax.device_put(a, sharding) for a in concat_in + concat_zeros]
    out_arrs = jax.block_until_ready(fn(*args))  # compile + first run
    times = []
    for _ in range(iters):
        t0 = time.perf_counter()
        out_arrs = jax.block_until_ready(fn(*args))
        times.append((time.perf_counter() - t0) * 1e9)
    qname_i = out_names.index("out_q")
    xname_i = out_names.index("out_xh")
    oq = np.asarray(out_arrs[qname_i]).reshape(NC, NO, 2, 128, 4, NTILE)
    ox = np.asarray(out_arrs[xname_i]).reshape(NC, 128, 3, KT, BPC)
    out = np.concatenate(
        [_reconstruct(oq[c], ox[c], Psih) for c in range(NC)], axis=0
    )
    return out, times


# revision 81
# speedup vs baseline: 1.4645x; 1.4241x over previous
"""CWT (GMW filterbank) Trainium2 kernel, v4.

Computes Wx = ifft(Psih * fft(reflect_pad(x)))[..., N1:N1+L] for
x (32, 2048) f32, Psih (256, 4096) f32 -> out (32, 256, 2048) complex64.

Strategy (8 NeuronCores, data-parallel over batch, 4 rows/core),
optimized for SINGLE-SHOT execution time (prologue included):
  - Forward FFT via two-stage Cooley-Tukey (4096 = 32 x 128): fp16
    stage-1 DFT-128 matmul, DVE twiddle, PE transposes, stage-2
    block-diagonal matmul (ONE 3x64-col matmul per att component via
    pre-ordered plane triplets) producing xh {re, im, -im}; the 1/4096
    ifft normalization is folded into the stage-2 weights.
  - Banded mirror inverse DFT: per (octave, k-tile) pair (29 pairs at
    the 1e-2 band threshold) four products U = Pre@Er, W = Pre@Ei,
    -V = (-Pim)@Ei, Z = Pim@Er over the LEFT half n in [1024, 2048)
    only, shipped RAW to the host as fp16 (U,W)/(-V,Z) quad planes.
    The host reconstructs left = (U-V) + i(W+Z) and the mirrored right
    half = (U+V) + i(Z-W) for free, and computes the n=2048 center
    column from the shipped 24 KB xh spectrum.  This halves the output
    DMA (8.4 MB/core) and deletes all mirror/interleave DVE work and
    the old per-octave ctr matmuls.
  - fp16 throughout (E scaled to +-1, banded Psih 237 KB, x, xh, P,
    quads): same PE/DMA cost as bf16, ~8x finer quantization
    (global rel err ~5.7e-4).
  - DMA schedule: inputs (const packs, xs, banded psih, 16 E k-tiles
    kt-ascending = first-use order) stream on the SP HWDGE queue; the
    first 6 quad outputs also ride the SP queue BEHIND the E tiles
    (FIFO = input priority, no fence needed), later quads go via the
    gpsimd SWDGE queue, and the final unit's halves take the idle
    SP/ACT HWDGE queues so no SWDGE descriptor-gen trails the end.
  - Octave order [5,4,6,3,2,7,1,0] (sim-swept): wide-ish octaves early (slow PE
    consumption while E streams in), narrow octaves interleaved so
    their PSUM drains hide under wide-octave matmul stretches, widest
    (o0) last so a single quad trails the final matmul; that last unit
    is split into column halves so its copy+DMA overlaps its matmuls.
  - P-gen (P = Psih (.) xh, 3 DVE ops/octave) runs ahead per a fixed
    lookahead schedule so the big o1/o0 P tiles land in DVE slack.
  - PE p-state warmup: memset-fed transposes at t=0 ramp the PE clock
    while the first DMAs are still in flight.

Build notes (hard-won):
  - bacc.Bacc() + nc.compile() required (multi-wait legalization).
  - DVE tensor_tensor reads at most ONE operand from PSUM; free-dim APs
    capped at 3D; fp16 packed all-SBUF ops run 2x (broadcast-innermost
    operands drop back to 1x).
  - Matmul moving APs are capped at 512 elements (no [2,512] fusing).
  - PSUM is 8 banks; uv/wz [128,2,512] f32 tiles are 2 banks each,
    pool bufs=2 fills all 8; forward tiles are carved from the same
    tags via rotation.
Measured: TimelineSim single-shot 64.0 us (baseline v3: 119.7); HW
steady-state (loop-amortized) 71.5-72 us/iter (v3: 102); global rel
err 5.7e-4 (v3: 2.9e-3).  P-gen runs at DVE 2x via the batch-repeated
banded Psih (per-octave tiles DMA'd just-in-time within the E stream);
non-final quads ship as ONE 512 KB DMA per unit (E-tile pairing was
tried and REGRESSES: coarser arrival granularity stalls the early
octaves; per-kt E loads are the right grain).
"""

import numpy as np
import ml_dtypes

import concourse.bass as bass
import concourse.bacc as bacc
import concourse.mybir as mybir
import concourse.tile as tile
from concourse.bass_utils import run_bass_kernel_spmd

F16 = np.float16

B = 32          # batch
L = 2048        # signal length
UP = 4096       # padded length
N1 = 1024       # left pad (slice offset)
NA = 256        # scales
NV = 32         # voices/octave
NO = 8          # octaves
KF = 2048       # used frequency bins (Psih==0 at k=0 and k>=2048)
NC = 8          # cores
BPC = B // NC   # batch rows per core (4)
KT = KF // 128  # k tiles (16)
NTILE = 512     # output columns per matmul (left half = 2 tiles)
N1CT = 32       # CT inner length  (n = n1 + 32*n2)
N2CT = 128      # CT outer length

_CACHE = {}
_ORDER = (5, 4, 6, 3, 2, 7, 1, 0)


def _bands_from(Psih):
    bands = []
    for o in range(NO):
        sub = np.asarray(Psih)[NV * o:NV * (o + 1), :KF]
        ks = np.nonzero((sub > 1e-2 * 2.0).any(axis=0))[0]
        bands.append((int(ks.min()) // 128, int(ks.max()) // 128 + 1))
    return bands


def _host_constants(Psih):
    """CT-FFT / inverse-DFT constant tensors + per-octave bands."""
    bands = _bands_from(Psih)

    # inverse DFT left half, NO 1/UP scale (folded into w32):
    # E[k, n] = exp(2i pi k n / UP), n in [N1, N1+L/2)
    kk = np.arange(KF)[:, None]
    nn = np.arange(N1, N1 + L // 2)[None, :]
    ph = 2.0 * np.pi * ((kk * nn) % UP) / UP
    # device layout: (kt, k_in 128, lnt, ri, n 512) fp16
    e_dev = np.empty((KT, 128, 2, 2, NTILE), dtype=F16)
    e_dev[:, :, :, 0, :] = np.cos(ph).reshape(KT, 128, 2, NTILE).astype(F16)
    e_dev[:, :, :, 1, :] = np.sin(ph).reshape(KT, 128, 2, NTILE).astype(F16)

    # stage-1 DFT-128 weights: W[n2, p] = exp(-2i pi n2 p / 128), fp16
    # (stage 1 runs fully in fp16: 1 cycle/row instead of 4)
    n2 = np.arange(N2CT)[:, None]
    p = np.arange(128)[None, :]
    w128_dev = np.empty((N2CT, 2, 128), dtype=F16)
    w128_dev[:, 0, :] = np.cos(2 * np.pi * n2 * p / N2CT).astype(F16)
    w128_dev[:, 1, :] = -np.sin(2 * np.pi * n2 * p / N2CT).astype(F16)

    # twiddle exp(-2i pi p n1 / UP): planes (cos, sin, -sin), f32
    pp = np.arange(128)[:, None]
    n1 = np.arange(N1CT)[None, :]
    tw_dev = np.empty((128, 3, N1CT), dtype=np.float32)
    tw_dev[:, 0, :] = np.cos(2 * np.pi * pp * n1 / UP)
    tw_dev[:, 1, :] = np.sin(2 * np.pi * pp * n1 / UP)
    tw_dev[:, 2, :] = -tw_dev[:, 1, :]

    # stage-2 block-diagonal rhs, scaled by 1/UP (ifft normalization):
    # R[(b',n1), plane, (b,q)] = (b'==b) * f(n1, q) / UP
    # planes ordered so each att component does ONE 3x64-col matmul:
    #   re-planes  (c, -s,  s) -> out blocks (Xre, Xim, -Xim)
    #   im-planes  (s,  c, -c)
    n1c = np.arange(N1CT)[:, None]
    qq = np.arange(KT)[None, :]
    c32 = np.cos(2 * np.pi * n1c * qq / N1CT) / UP
    s32 = np.sin(2 * np.pi * n1c * qq / N1CT) / UP
    w32_dev = np.zeros((BPC * N1CT, 6, BPC * KT), dtype=F16)
    for b in range(BPC):
        sl_r = slice(b * N1CT, (b + 1) * N1CT)
        sl_c = slice(b * KT, (b + 1) * KT)
        for pl, m in enumerate((c32, -s32, s32, s32, c32, -c32)):
            w32_dev[sl_r, pl, sl_c] = m.astype(F16)

    id128_dev = np.eye(128, dtype=np.float32)

    return e_dev, w128_dev, tw_dev, w32_dev, id128_dev, bands


def _pack_psihb(Psih, bands):
    """Banded Psih, fp16, repeated over the batch dim: [128 (k_in),
    sum(nk)*NV*BPC] with per-octave slices laid out [nk, NV, BPC]
    (b innermost) so every P-gen operand is packed fp16 -> DVE 2x."""
    tot = sum(hi - lo for lo, hi in bands)
    psihb = np.empty((128, tot * NV * BPC), dtype=F16)
    off = 0
    for o, (lo, hi) in enumerate(bands):
        nk = hi - lo
        # [nk, 128, NV] <- Psih[a, k].T slices
        blk = np.asarray(Psih)[NV * o:NV * (o + 1),
                               lo * 128:hi * 128].T.reshape(nk, 128, NV)
        rep = np.repeat(
            blk.transpose(1, 0, 2).reshape(128, nk * NV), BPC, axis=1
        )
        psihb[:, off * NV * BPC:(off + nk) * NV * BPC] = rep.astype(F16)
        off += nk
    return psihb


def _build_program(e_dev, w128_dev, tw_dev, w32_dev, id128_dev,
                   bands, reps=1, variant="full"):
    f32 = mybir.dt.float32
    f16 = mybir.dt.float16

    tot = sum(hi - lo for lo, hi in bands)
    offs = {}
    off = 0
    for o, (lo, hi) in enumerate(bands):
        offs[o] = off
        off += hi - lo

    nc = bacc.Bacc()
    xp_in = nc.dram_tensor("xp", [BPC, UP], f16, kind="ExternalInput")
    psihb_in = nc.dram_tensor("psihb", [128, tot * NV * BPC], f16,
                              kind="ExternalInput")
    outq_t = nc.dram_tensor("out_q", [NO, 2, 128, 4, NTILE], f16,
                            kind="ExternalOutput")
    xh_t = nc.dram_tensor("out_xh", [128, 3, KT, BPC], f16,
                          kind="ExternalOutput")

    e_c = nc.inline_tensor(e_dev, name="econst")
    c32_dev = np.concatenate(
        [id128_dev, tw_dev.reshape(128, 96)], axis=1
    )
    c16_dev = np.concatenate(
        [w128_dev.reshape(128, 256), w32_dev.reshape(128, 384)], axis=1
    )
    c32_c = nc.inline_tensor(c32_dev, name="c32const")
    c16_c = nc.inline_tensor(c16_dev, name="c16const")

    with tile.TileContext(nc) as tc:
        with (
            tc.tile_pool(name="persist", bufs=1) as persist,
            tc.tile_pool(name="pfix", bufs=1) as pfix,
            tc.tile_pool(name="fwd", bufs=2) as fwdp,
            tc.tile_pool(name="stg", bufs=16) as stgp,
            tc.tile_pool(name="stgf", bufs=2) as stgf,
            tc.tile_pool(name="ps_m", bufs=2, space="PSUM") as ps_m,
        ):
            # ---- PE p-state warmup: zeros tile via DVE memset (no DMA
            # dependency), then transposes keep PE busy and ramping while
            # the prologue DMAs stream in ----
            z_sb = persist.tile([128, 128], f32, tag="zwarm")
            nc.vector.memset(z_sb[:], 0.0)
            dummy = ps_m.tile([128, 2, NTILE], f32, tag="wz", name="dmy")
            # 6 only: on real HW these cost ~440 ns each with no
            # measurable DVFS benefit (measured via in-loop fillers), so
            # keep the warmup safely inside the first-DMA wait window
            for _ in range(6):
                nc.tensor.transpose(dummy[:, 0, 0:128], z_sb, z_sb)

            # ---- prologue: all input DMAs on the SP HWDGE queue, in
            # first-use order; small consts packed per dtype into single
            # DMAs; E k-tiles ascending (= first-use order for the
            # wide-early octave schedule). ----
            c16_sb = persist.tile([128, 640], f16, tag="c16")
            nc.sync.dma_start(out=c16_sb, in_=c16_c[:])
            w128_sb = c16_sb[:, 0:256].rearrange("p (r q) -> p r q", r=2)
            w32_sb = c16_sb[:, 256:640].rearrange("p (r q) -> p r q", r=6)
            xs_sb = persist.tile([N2CT, BPC * N1CT], f16, tag="xs")
            nc.sync.dma_start(
                out=xs_sb.rearrange("p (b m) -> p b m", b=BPC),
                in_=xp_in[:].rearrange("b (n2 n1) -> n2 b n1", n1=N1CT),
            )
            c32_sb = persist.tile([128, 224], f32, tag="c32")
            nc.sync.dma_start(out=c32_sb, in_=c32_c[:])
            id_sb = c32_sb[:, 0:128]
            tw_sb = c32_sb[:, 128:224].rearrange("p (r m) -> p r m", r=3)
            # per-octave psihb tiles, DMAs interleaved into the E
            # stream so each arrives just before its P-gen needs it
            # (separate tiles so a P-gen waits only its own slice)
            psih_tiles = {}

            def _psihb_dma(o):
                lo, hi = bands[o]
                nk = hi - lo
                offc = offs[o] * NV * BPC
                ptile = persist.tile([128, nk * NV * BPC], f16,
                                     tag=f"psihb{o}")
                nc.sync.dma_start(
                    out=ptile, in_=psihb_in[:, offc:offc + nk * NV * BPC]
                )
                psih_tiles[o] = ptile

            psihb_at = {-1: [5, 4], 0: [6, 3], 1: [7, 2], 2: [1], 3: [0]}
            for o in psihb_at[-1]:
                _psihb_dma(o)
            etiles = {}
            for kt in range(KT):
                et = persist.tile([128, 2, 2, NTILE], f16, tag=f"e{kt}")
                nc.sync.dma_start(out=et, in_=e_c[kt])
                etiles[kt] = et
                for o in psihb_at.get(kt, ()):
                    _psihb_dma(o)

            ctx = dict(
                nc=nc, bands=bands, offs=offs, outq_t=outq_t, xh_t=xh_t,
                z_sb=z_sb, dummy=dummy,
                persist=persist, pfix=pfix, fwdp=fwdp, stgp=stgp,
                stgf=stgf, ps_m=ps_m,
                psih_tiles=psih_tiles, xs_sb=xs_sb, w128_sb=w128_sb, tw_sb=tw_sb,
                w32_sb=w32_sb, id_sb=id_sb, etiles=etiles,
                f32=f32, f16=f16, variant=variant,
            )

            if reps == 1:
                _emit_body(ctx)
            else:
                with tc.For_i(0, reps, 1):
                    _emit_body(ctx)
    nc.compile()
    return nc


def _neg_comp(apx, n):
    """Same AP with dim 1 read in reverse order (indices n-1 .. 0)."""
    return bass.AP(
        apx.tensor,
        apx.offset + (n - 1) * apx.ap[1][0],
        [list(apx.ap[0]), [-apx.ap[1][0], n]] + [list(d) for d in apx.ap[2:]],
    )


def _emit_fwd(ctx):
    """Forward CT-FFT (4096 = 32 x 128): xh_all[p, {re,im,-im}, q, b] fp16,
    scaled by 1/UP (via w32)."""
    nc = ctx["nc"]
    f32, f16 = ctx["f32"], ctx["f16"]
    ps_m, fwdp, persist = ctx["ps_m"], ctx["fwdp"], ctx["persist"]
    xs_sb, w128_sb, tw_sb = ctx["xs_sb"], ctx["w128_sb"], ctx["tw_sb"]
    w32_sb, id_sb = ctx["w32_sb"], ctx["id_sb"]
    mult = mybir.AluOpType.mult

    # stage 1: A[p, (b, n1)] = sum_n2 xs[n2, (b, n1)] W128[n2, p], f32
    a_ps = ps_m.tile([128, 2, NTILE], f32, tag="uv", name="aps")
    for ri in range(2):
        nc.tensor.matmul(
            a_ps[:, ri, 0:BPC * N1CT], w128_sb[:, ri, :], xs_sb,
            start=True, stop=True,
        )

    # twiddle At = A * exp(-2i pi p n1/4096), 3 DVE ops via -sin plane
    tmp = fwdp.tile([128, 4, BPC, N1CT], f32, tag="twtmp")
    at = fwdp.tile([128, 2, BPC * N1CT], f32, tag="at")
    a2 = a_ps[:, :, 0:BPC * N1CT].rearrange("p r (b m) -> p r b m", b=BPC)
    twc = tw_sb[:, 0, :][:, None, None, :].to_broadcast((128, 2, BPC, N1CT))
    tws = tw_sb[:, 1:3, :][:, :, None, :].to_broadcast((128, 2, BPC, N1CT))
    nc.vector.tensor_tensor(tmp[:, 0:2], a2, twc, mult)
    nc.vector.tensor_tensor(tmp[:, 2:4], a2, tws, mult)
    nc.vector.tensor_sub(
        at.rearrange("p r (b m) -> p r b m", b=BPC),
        tmp[:, 0:2], _neg_comp(tmp[:, 2:4], 2),
    )

    # transpose to [(b, n1), p]; round to fp16 for stage 2
    ta_ps = ps_m.tile([128, 2, NTILE], f32, tag="wz", name="taps")
    nc.tensor.transpose(ta_ps[:, 0, 0:128], at[:, 0, :], id_sb)
    nc.tensor.transpose(ta_ps[:, 1, 0:128], at[:, 1, :], id_sb)
    att = fwdp.tile([128, 2, 128], f16, tag="att")
    nc.vector.tensor_copy(out=att, in_=ta_ps[:, :, 0:128])

    # stage 2: XH[p, {re,im,-im}, (b, q)] -- two 3x64-col matmuls, one
    # per att component, using the pre-ordered w32 plane triplets
    xh_ps = ps_m.tile([128, 2, NTILE], f32, tag="uv", name="xhps")
    nq = BPC * KT
    nc.tensor.matmul(xh_ps[:, 0, 0:3 * nq], att[:, 0, :],
                     w32_sb[:, 0:3, :], start=True, stop=False)
    nc.tensor.matmul(xh_ps[:, 0, 0:3 * nq], att[:, 1, :],
                     w32_sb[:, 3:6, :], start=False, stop=True)
    # xh_all[p, comp, q, b] fp16 in SBUF for the P-gen broadcasts
    xh_all = persist.tile([128, 3, KT, BPC], f16, tag="xh")
    nc.vector.tensor_copy(
        out=xh_all,
        in_=xh_ps[:, 0, 0:3 * nq].rearrange("p (r b q) -> p r q b",
                                            r=3, b=BPC),
    )
    ctx["xh_all"] = xh_all
    ctx["xh_ps"] = xh_ps
    # ship the (tiny) spectrum: host computes the n=2048 center column
    # directly from it (emitted here, but the SP queue FIFO parks it
    # behind the E-tile loads, where it belongs)
    nc.sync.dma_start(out=ctx["xh_t"][:], in_=xh_all)


def _emit_pgen(ctx, o, eng=None, from_psum=False):
    """P[(o, kt in band, {re, im, -im})] = Psih (.) xh, 3 ops/octave.

    eng: engine to run on (default DVE); gpsimd for octaves generated
    while the Pool engine is otherwise idle.  from_psum: read xh straight
    from the stage-2 PSUM tile (skips the xh_all copy latency; only legal
    for the FIRST octave, before the PSUM tile rotates away)."""
    nc, bands, offs = ctx["nc"], ctx["bands"], ctx["offs"]
    pfix = ctx["pfix"]
    f16 = ctx["f16"]
    if eng is None:
        eng = nc.vector
    klo, khi = bands[o]
    nk = khi - klo
    # one tile PER COMPONENT so a matmul waits only the component it
    # reads (comp 2 generated first: it feeds the first matmul emitted
    # for single-ktile octaves)
    psih_ap = (
        ctx["psih_tiles"][o][:]
        .rearrange("p (k a b) -> p k a b", a=NV, b=BPC)
    )
    ptc = {}
    for comp in (2, 1, 0):
        pt = pfix.tile([128, nk, NV * BPC], f16, tag=f"P{o}c{comp}")
        out_ap = pt.rearrange("p k (a b) -> p k a b", b=BPC)
        xh_ap = (
            ctx["xh_all"][:, comp, klo:khi, None, :]
            .to_broadcast((128, nk, NV, BPC))
        )
        eng.tensor_tensor(out_ap, psih_ap, xh_ap, mybir.AluOpType.mult)
        ptc[comp] = pt
    ctx.setdefault("P", {})[o] = ptc


def _emit_body(ctx):
    """Forward + P-gen + banded quad inverse + quad output DMAs."""
    nc, bands = ctx["nc"], ctx["bands"]
    outq_t = ctx["outq_t"]
    stgp, stgf, ps_m = ctx["stgp"], ctx["stgf"], ctx["ps_m"]
    etiles = ctx["etiles"]
    f32, f16 = ctx["f32"], ctx["f16"]

    _emit_fwd(ctx)

    # Narrow octaves interleaved between wide ones so their copy+DMA
    # drains hide under wide-octave matmul stretches; widest (o0) last
    # so only one quad trails the final matmul.  (Order validated by a
    # sim sweep; P-gen runs two octaves ahead of the matmul stream so
    # the big o1/o0 P tiles are generated during wide-octave DVE slack.)
    order = list(_ORDER)
    _emit_pgen(ctx, order[0])
    _emit_pgen(ctx, order[1])
    pgen_after = {i: [order[i + 2]] for i in range(NO - 2)}

    ucnt = 0
    for oi, o in enumerate(order):
        klo, khi = bands[o]
        kts = list(range(klo, khi))
        ptc = ctx["P"][o]

        def P(comp, kt):
            return ptc[comp][:, kt - klo, :]

        # The very last unit is split into column halves so its copy+DMA
        # tail overlaps its own matmuls; its out-DMAs go on the SP/ACT
        # HWDGE queues (no SWDGE descriptor-gen serialization at the end).
        final = (oi == NO - 1)
        halves = ((slice(0, 256), slice(256, 512)) if final
                  else (slice(0, NTILE),))

        for lnt in range(2):
            for hs in (halves if (final and lnt == 1) else (slice(0, NTILE),)):
                # PSUM tiles pair products sharing the stationary weight:
                # uw = (U, W) from P0 (er then ei, one weight load on hw);
                # vz = (-V, Z) from P2/P1.  (A single [2,512] matmul is
                # illegal: matmul moving APs cap at 512 elements.)
                uv = ps_m.tile([128, 2, NTILE], f32, tag="uv")
                wz = ps_m.tile([128, 2, NTILE], f32, tag="wz")
                for j, kt in enumerate(kts):
                    first, last = (j == 0), (j == len(kts) - 1)
                    er = etiles[kt][:, lnt, 0, hs]
                    ei = etiles[kt][:, lnt, 1, hs]
                    if not last:
                        nc.tensor.matmul(uv[:, 0, hs], P(0, kt), er,
                                         start=first, stop=False)
                        nc.tensor.matmul(uv[:, 1, hs], P(0, kt), ei,
                                         start=first, stop=False)
                        nc.tensor.matmul(wz[:, 0, hs], P(2, kt), ei,
                                         start=first, stop=False)
                        nc.tensor.matmul(wz[:, 1, hs], P(1, kt), er,
                                         start=first, stop=False)
                    else:
                        # vz groups stop first so the slower DVE copy
                        # starts before the ACT one
                        nc.tensor.matmul(wz[:, 0, hs], P(2, kt), ei,
                                         start=first, stop=True)
                        nc.tensor.matmul(wz[:, 1, hs], P(1, kt), er,
                                         start=first, stop=True)
                        nc.tensor.matmul(uv[:, 0, hs], P(0, kt), er,
                                         start=first, stop=True)
                        nc.tensor.matmul(uv[:, 1, hs], P(0, kt), ei,
                                         start=first, stop=True)
                # quad (U,W,-V,Z) to SBUF fp16.  Non-final units ship as
                # ONE DMA per unit (fewer instructions; the 16-buf pool
                # hides the wait-for-both-copies); the final unit keeps
                # split halves so each ships as soon as its copy lands.
                # The first 6 units ship on the SP queue BEHIND the E
                # tiles (FIFO = input priority); later units go via the
                # gpsimd SWDGE queue (input stream nearly done by then).
                ucnt += 1
                if final and lnt == 1:
                    quv = stgf.tile([128, 2, NTILE], f16, tag="quv")
                    qwz = stgf.tile([128, 2, NTILE], f16, tag="qwz")
                    nc.scalar.copy(out=quv[:, :, hs], in_=uv[:, :, hs])
                    nc.scalar.dma_start(out=outq_t[o, lnt, :, 0:2, hs],
                                        in_=quv[:, :, hs])
                    nc.vector.tensor_copy(out=qwz[:, :, hs], in_=wz[:, :, hs])
                    nc.sync.dma_start(out=outq_t[o, lnt, :, 2:4, hs],
                                      in_=qwz[:, :, hs])
                else:
                    q = stgp.tile([128, 4, NTILE], f16, tag="quad")
                    nc.scalar.copy(out=q[:, 0:2, hs], in_=uv[:, :, hs])
                    nc.vector.tensor_copy(out=q[:, 2:4, hs], in_=wz[:, :, hs])
                    if ucnt <= 6:
                        nc.sync.dma_start(out=outq_t[o, lnt, :, :, hs],
                                          in_=q[:, :, hs])
                    else:
                        nc.gpsimd.dma_start(out=outq_t[o, lnt, :, :, hs],
                                            in_=q[:, :, hs])

        # P-gen for upcoming octaves per the lookahead schedule
        for oo in pgen_after.get(oi, ()):
            _emit_pgen(ctx, oo)


def _get_program(Psih, reps=1, variant="full"):
    key = f"prog{reps}_{variant}"
    if key not in _CACHE:
        if "consts" not in _CACHE:
            _CACHE["consts"] = _host_constants(np.asarray(Psih))
        (e_dev, w128_dev, tw_dev, w32_dev, id128_dev,
         bands) = _CACHE["consts"]
        nc = _build_program(e_dev, w128_dev, tw_dev, w32_dev,
                            id128_dev, bands, reps=reps, variant=variant)
        _CACHE[key] = (nc, bands)
    return _CACHE[key]


def _reflect_pad(x):
    return np.pad(x, ((0, 0), (N1, UP - L - N1)), mode="reflect")


_CTRW = {}


def _ctr_weight(Psih):
    """A[a, k] = Psih[a, k] * (-1)^k for the host-side n=2048 column."""
    if "w" not in _CTRW:
        sign = ((-1.0) ** (np.arange(KF) % 2)).astype(np.float32)
        _CTRW["w"] = np.asarray(Psih)[:, :KF].astype(np.float32) * sign
    return _CTRW["w"]


def _reconstruct(outq, xh, Psih):
    """Host-side: quads [NO, 2, 128, 4, 512] fp16 + spectrum
    xh [128, 3, KT, BPC] fp16 -> (BPC, NA, L) complex64 for one core."""
    oq = np.asarray(outq).astype(np.float32)
    # rows p = b*NV + a (b-major)
    oq = oq.reshape(NO, 2, NV, BPC, 4, NTILE)
    U = oq[:, :, :, :, 0]
    W = oq[:, :, :, :, 1]
    nV = oq[:, :, :, :, 2]
    Z = oq[:, :, :, :, 3]
    left = (U + nV) + 1j * (W + Z)        # [o, lnt, a, b, n]
    right = (U - nV) + 1j * (Z - W)
    # -> [b, o, a, lnt*512+n]
    left = left.transpose(3, 0, 2, 1, 4).reshape(BPC, NO * NV, L // 2)
    right = right.transpose(3, 0, 2, 1, 4).reshape(BPC, NO * NV, L // 2)
    out = np.empty((BPC, NA, L), dtype=np.complex64)
    out[:, :, 0:L // 2] = left
    # mirror: col 2048 - n2 for n2 in [1, 1024)
    out[:, :, L // 2 + 1:] = right[:, :, 1:][:, :, ::-1]
    # n=2048 center column from the shipped spectrum:
    # ctr[b, a] = sum_k Psih[a,k] * xh[b,k] * (-1)^k   (xh includes 1/UP)
    xh = np.asarray(xh).astype(np.float32)       # [p, comp, q, b]
    xhc = (xh[:, 0] + 1j * xh[:, 1]).transpose(2, 1, 0).reshape(BPC, KF)
    out[:, :, L // 2] = xhc @ _ctr_weight(Psih).T.astype(np.complex64)
    return out


def kernel(x, Psih=None, **_unused):
    x = np.ascontiguousarray(np.asarray(x), dtype=np.float32)
    if Psih is None:
        raise ValueError("Psih input required")
    nc, bands = _get_program(Psih)
    psihb = _pack_psihb(Psih, bands)
    xp = np.ascontiguousarray(_reflect_pad(x).astype(F16))
    in_maps = [
        {"xp": np.ascontiguousarray(xp[BPC * c:BPC * (c + 1)]),
         "psihb": psihb}
        for c in range(NC)
    ]
    res = run_bass_kernel_spmd(nc, in_maps, core_ids=list(range(NC)))
    out = np.concatenate(
        [_reconstruct(r["out_q"], r["out_xh"], Psih) for r in res.results],
        axis=0,
    )
    return out


def bench(x, Psih, iters=20, reps=1, variant="full"):
    """Run the kernel repeatedly on-device; returns (out_complex, times_ns).

    Builds the same shard_map executable as bass2jax.run_bass_via_pjrt but
    without donation, so the warm executable can be re-invoked with
    device-resident inputs."""
    import time
    import jax
    from jax.sharding import Mesh, PartitionSpec
    from jax.experimental.shard_map import shard_map
    from concourse import bass2jax

    x = np.ascontiguousarray(np.asarray(x), dtype=np.float32)
    nc, bands = _get_program(Psih, reps=reps, variant=variant)
    psihb = _pack_psihb(Psih, bands)
    bass2jax.install_neuronx_cc_hook()

    part_name = nc.partition_id_tensor.name if nc.partition_id_tensor else None
    in_names, out_names, out_avals = [], [], []
    for alloc in nc.m.functions[0].allocations:
        if not isinstance(alloc, mybir.MemoryLocationSet):
            continue
        name = alloc.memorylocations[0].name
        if alloc.kind == "ExternalInput":
            if name != part_name:
                in_names.append(name)
        elif alloc.kind == "ExternalOutput":
            out_names.append(name)
            out_avals.append(
                jax.core.ShapedArray(
                    tuple(alloc.tensor_shape), mybir.dt.np(alloc.dtype)
                )
            )
    n_params = len(in_names)
    all_names = in_names + out_names
    if part_name is not None:
        all_names = all_names + [part_name]

    def _body(*args):
        operands = list(args)
        if part_name is not None:
            operands.append(bass2jax.partition_id_tensor())
        outs = bass2jax._bass_exec_p.bind(
            *operands,
            out_avals=tuple(out_avals),
            in_names=tuple(all_names),
            out_names=tuple(out_names),
            lowering_input_output_aliases=(),
            sim_require_finite=True,
            sim_require_nnan=True,
            nc=nc,
        )
        return tuple(outs)

    devices = jax.devices()[:NC]
    mesh = Mesh(np.asarray(devices), ("core",))
    nin = n_params + len(out_names)
    fn = jax.jit(
        shard_map(
            _body,
            mesh=mesh,
            in_specs=(PartitionSpec("core"),) * nin,
            out_specs=(PartitionSpec("core"),) * len(out_names),
            check_rep=False,
        ),
        keep_unused=True,
    )
    xp = np.ascontiguousarray(_reflect_pad(x).astype(F16))
    in_map = {"xp": xp, "psihb": np.concatenate([psihb] * NC, axis=0)}
    concat_in = [in_map[n] for n in in_names]
    concat_zeros = [
        np.zeros((NC * a.shape[0], *a.shape[1:]), a.dtype) for a in out_avals
    ]
    sharding = jax.sharding.NamedSharding(mesh, PartitionSpec("core"))
    args = [jax.device_put(a, sharding) for a in concat_in + concat_zeros]
    out_arrs = jax.block_until_ready(fn(*args))  # compile + first run
    times = []
    for _ in range(iters):
        t0 = time.perf_counter()
        out_arrs = jax.block_until_ready(fn(*args))
        times.append((time.perf_counter() - t0) * 1e9)
    qname_i = out_names.index("out_q")
    xname_i = out_names.index("out_xh")
    oq = np.asarray(out_arrs[qname_i]).reshape(NC, NO, 2, 128, 4, NTILE)
    ox = np.asarray(out_arrs[xname_i]).reshape(NC, 128, 3, KT, BPC)
    out = np.concatenate(
        [_reconstruct(oq[c], ox[c], Psih) for c in range(NC)], axis=0
    )
    return out, times
